# revision 1
# baseline (speedup 1.0000x reference)
"""BatchGAT (2-layer, 8-head GAT over 32 graphs of 512 nodes) on 8 TRN2 NeuronCores.

Data-parallel over the batch: each core processes 4 graphs. Per graph/layer the
masked softmax attention is built in transposed layout E^T[j, i] (j = neighbor on
partitions) so the aggregation A @ hp runs on TensorE with E^T as lhsT and hp
(plus a ones column) as rhs — the softmax denominator falls out as an output
column and normalization is a per-partition scale fused into PSUM evacuation.

E^T[j,i] = exp(leaky_relu(s_i + d_j))*adj is built by two engine paths, load
balanced across Vector / GpSimd / Scalar / Tensor:

 D-path (DVE): exp(leaky(x)) = max(es_i*r_j, es2_i) * ed2_j with es=exp(s),
   es2=exp(0.2 s) broadcast row tiles, r=exp(0.8 d), ed2=exp(0.2 d) per-partition
   scalars; the ed2 factor is folded into a cheap per-head rescale of the [128,65]
   hp tile (it rides through the aggregation), and the adjacency mask is applied
   as min(u, adjP) with adjP in {BIG, 0}.

 A-path (ACT): logits x = s_i + d_j via a K=2 matmul, plus an identity-weight
   matmul accumulating an additive mask adjM in {0, -BIG} into the same PSUM;
   then E = max(exp(x), exp(0.2 x)) — two ACT exps from PSUM (one table set) and
   one DVE max.
"""

import sys

if "/opt/trn_rl_repo" not in sys.path:
    sys.path.insert(0, "/opt/trn_rl_repo")

import numpy as np
import ml_dtypes

import concourse.bacc as bacc
import concourse.mybir as mybir
from concourse import tile
from concourse.bass_utils import run_bass_kernel_spmd
from concourse.alu_op_type import AluOpType

F32 = mybir.dt.float32
BF16 = mybir.dt.bfloat16
BF = ml_dtypes.bfloat16
AF = mybir.ActivationFunctionType

B, N, FIN, H, F = 32, 512, 64, 8, 64
NCORES = 8
G = B // NCORES          # graphs per core
NT = N // 128            # node tiles
C1 = H * F               # layer-1 input features (512)
BIG = 30000.0

_cached = {}


def _a_path(h, jt):
    """Which (head, j-tile) pairs use the ACT/PE logit path."""
    return ((h * NT + jt) * 5) % 16 < 5


def _gps_mask(h, jt):
    """Which D-pairs run the mask mult on GpSimd."""
    return False


def _build():
    nc = bacc.Bacc("TRN2", target_bir_lowering=False, debug=False)

    xT = nc.dram_tensor("xT", [G, FIN, N], F32, kind="ExternalInput").ap()
    adjP = nc.dram_tensor("adjP", [G, N, N], BF16, kind="ExternalInput").ap()  # {0,1}
    adjM = nc.dram_tensor("adjM", [G, N, N], BF16, kind="ExternalInput").ap()
    ident = nc.dram_tensor("ident", [128, 128], BF16, kind="ExternalInput").ap()
    w0d = nc.dram_tensor("w0d", [FIN, F + 2 * H], F32, kind="ExternalInput").ap()
    w1d = nc.dram_tensor("w1d", [C1, F + 2 * H], BF16, kind="ExternalInput").ap()
    out = nc.dram_tensor("out", [G, N, F], F32, kind="ExternalOutput").ap()

    with tile.TileContext(nc) as tc:
        _emit(nc, tc, xT, adjP, adjM, ident, w0d, w1d, out)
    nc.compile()
    return nc


def _emit(nc, tc, xT, adjP, adjM, ident, w0d, w1d, out):
    from contextlib import ExitStack

    ctx = ExitStack()
    with ctx:
        # weights: [W | W@a_dst | W@a_src] -> proj matmul yields [hp | d | s]
        wpool = ctx.enter_context(tc.tile_pool(name="weights", bufs=1))
        w0d_sb = wpool.tile([FIN, F + 2 * H], F32, tag="w0d")
        nc.sync.dma_start(w0d_sb[:], w0d[:])
        w1d_sb = wpool.tile([128, NT, F + 2 * H], BF16, tag="w1d")
        nc.sync.dma_start(w1d_sb[:], w1d.rearrange("(c p) f -> p c f", p=128))
        id_sb = wpool.tile([128, 128], BF16, tag="ident")
        nc.sync.dma_start(id_sb[:], ident[:])
        ones_flat = wpool.tile([1, H * N], BF16, tag="ones_flat")
        nc.vector.memset(ones_flat[:], 1.0)

        xt_pool = ctx.enter_context(tc.tile_pool(name="xt", bufs=3))
        adj_pool = ctx.enter_context(tc.tile_pool(name="adj", bufs=3 * NT))
        hp_pool = ctx.enter_context(tc.tile_pool(name="hp", bufs=3 * NT))
        hps_pool = ctx.enter_context(tc.tile_pool(name="hps", bufs=2 * NT))
        dsc_pool = ctx.enter_context(tc.tile_pool(name="dscal", bufs=3 * NT))
        esr_pool = ctx.enter_context(tc.tile_pool(name="esr", bufs=3))
        esbc_pool = ctx.enter_context(tc.tile_pool(name="esbc", bufs=20))
        lg_pool = ctx.enter_context(tc.tile_pool(name="lg", bufs=2))
        et_pool = ctx.enter_context(tc.tile_pool(name="et", bufs=2 * NT))
        u_pool = ctx.enter_context(tc.tile_pool(name="u", bufs=8))
        den_pool = ctx.enter_context(tc.tile_pool(name="den", bufs=6))
        x1_pool = ctx.enter_context(tc.tile_pool(name="x1", bufs=3 * NT))
        x1t_pool = ctx.enter_context(tc.tile_pool(name="x1t", bufs=3 * NT))
        post_pool = ctx.enter_context(tc.tile_pool(name="post", bufs=4))
        out_pool = ctx.enter_context(tc.tile_pool(name="out", bufs=4))
        esd_pool = ctx.enter_context(tc.tile_pool(name="esd", bufs=3, space="DRAM"))

        ps_proj = ctx.enter_context(tc.tile_pool(name="ps_proj", bufs=2, space="PSUM"))
        ps_s = ctx.enter_context(tc.tile_pool(name="ps_s", bufs=1, space="PSUM"))
        ps_agg = ctx.enter_context(tc.tile_pool(name="ps_agg", bufs=3, space="PSUM"))
        ps_lg = ctx.enter_context(tc.tile_pool(name="ps_lg", bufs=2, space="PSUM"))

        graphs = {}

        def prologue(g, layer):
            st = {}
            if layer == 0:
                xt = xt_pool.tile([FIN, N], F32, tag="xt", name=f"xt_{g}")
                nc.gpsimd.dma_start(xt[:], xT[g])
                adjp_t, adjm_t = [], []
                for jt in range(NT):
                    ap_ = adj_pool.tile([128, N], BF16, tag="adjp",
                                        name=f"adjp_{g}_{jt}")
                    nc.gpsimd.dma_start(
                        ap_[:], adjP[g, jt * 128:(jt + 1) * 128, :])
                    adjp_t.append(ap_)
                    am_ = adj_pool.tile([128, N], BF16, tag="adjm",
                                        name=f"adjm_{g}_{jt}")
                    nc.gpsimd.dma_start(
                        am_[:], adjM[g, jt * 128:(jt + 1) * 128, :])
                    adjm_t.append(am_)
                graphs[g] = dict(xt=xt, adjp=adjp_t, adjm=adjm_t)
            gs = graphs[g]
            xt = gs["xt"]
            x1t = gs.get("x1t")

            # ---- projections: [hp | d | s] per node tile ----
            hp_aug, r_sc, ed2_sc = [], [], []
            for jt in range(NT):
                pp = ps_proj.tile([128, F + 2 * H], F32, tag="proj",
                                  name=f"pp_{g}_{layer}_{jt}")
                if layer == 0:
                    nc.tensor.matmul(
                        pp[:], xt[:, jt * 128:(jt + 1) * 128], w0d_sb[:],
                        start=True, stop=True)
                else:
                    for ct in range(NT):
                        nc.tensor.matmul(
                            pp[:], x1t[ct][:, jt * 128:(jt + 1) * 128],
                            w1d_sb[:, ct, :],
                            start=(ct == 0), stop=(ct == NT - 1))
                ha = hp_pool.tile([128, F + 1], BF16, tag="hp",
                                  name=f"ha_{g}_{layer}_{jt}")
                nc.scalar.copy(ha[:, 0:F], pp[:, 0:F])
                # ones column; 8.0 on layer 1 folds the head-mean into 1/den
                nc.vector.memset(ha[:, F:F + 1], 1.0 if layer == 0 else 8.0)
                hp_aug.append(ha)
                rr = dsc_pool.tile([128, H], F32, tag="rsc",
                                   name=f"rr_{g}_{layer}_{jt}")
                nc.scalar.activation(rr[:], pp[:, F:F + H], AF.Exp, scale=0.8)
                ee = dsc_pool.tile([128, H], F32, tag="ed2",
                                   name=f"ee_{g}_{layer}_{jt}")
                nc.scalar.activation(ee[:], pp[:, F:F + H], AF.Exp, scale=0.2)
                r_sc.append(rr)
                ed2_sc.append(ee)

            # s rows at partitions 0-7, raw d^T rows at partitions 32-39
            psd = ps_s.tile([32 + H, N], F32, tag="s",
                            name=f"psd_{g}_{layer}")
            ps = psd[0:H, :]
            pd = psd[32:32 + H, :]
            if layer == 0:
                nc.tensor.matmul(ps, w0d_sb[:, F + H:F + 2 * H], xt[:],
                                 start=True, stop=True)
                nc.tensor.matmul(pd, w0d_sb[:, F:F + H], xt[:],
                                 start=True, stop=True)
            else:
                for ct in range(NT):
                    nc.tensor.matmul(
                        ps, w1d_sb[:, ct, F + H:F + 2 * H], x1t[ct][:],
                        start=(ct == 0), stop=(ct == NT - 1))
                for ct in range(NT):
                    nc.tensor.matmul(
                        pd, w1d_sb[:, ct, F:F + H], x1t[ct][:],
                        start=(ct == 0), stop=(ct == NT - 1))

            # rows: exp(s), exp(.2 s), raw s, raw d -> DRAM bounce
            esr = esr_pool.tile([H, 4 * N], BF16, tag="esr",
                                name=f"esr_{g}_{layer}")
            nc.scalar.activation(esr[:, 0:N], ps, AF.Exp, scale=1.0)
            nc.scalar.activation(esr[:, N:2 * N], ps, AF.Exp, scale=0.2)
            nc.scalar.copy(esr[:, 2 * N:3 * N], ps)
            nc.scalar.copy(esr[:, 3 * N:4 * N], pd)
            esd = esd_pool.tile([H, 4 * N], BF16, tag="esd",
                                name=f"esd_{g}_{layer}")
            nc.gpsimd.dma_start(esd[:], esr[:])
            # flat 2-partition logit operands at base partition 0:
            # lgl = [ones; d-flat] (lhsT rows), lgr = [s-flat; ones] (rhs)
            lgl = lg_pool.tile([2, H * N], BF16, tag="lgl",
                                name=f"lgl_{g}_{layer}")
            nc.gpsimd.dma_start(lgl[0:1, :], ones_flat[:])
            nc.gpsimd.dma_start(lgl[1:2, :], esd[:, 3 * N:4 * N])
            lgr = lg_pool.tile([2, H * N], BF16, tag="lgr",
                                name=f"lgr_{g}_{layer}")
            nc.gpsimd.dma_start(lgr[0:1, :], esd[:, 2 * N:3 * N])
            nc.gpsimd.dma_start(lgr[1:2, :], ones_flat[:])

            es_bc, es2_bc = [], []
            for h in range(H):
                eb = esbc_pool.tile([128, 2 * N], BF16, tag="esbc",
                                    name=f"esbc_{g}_{layer}_{h}")
                nc.gpsimd.dma_start(
                    eb[:], esd[h:h + 1, 0:2 * N].partition_broadcast(128))
                es_bc.append(eb[:, 0:N])
                es2_bc.append(eb[:, N:2 * N])

            st.update(hp_aug=hp_aug, r_sc=r_sc, ed2_sc=ed2_sc, lgl=lgl,
                      lgr=lgr, es_bc=es_bc, es2_bc=es2_bc)
            return st

        def main(g, layer, st):
            gs = graphs[g]
            adjp_t, adjm_t = gs["adjp"], gs["adjm"]
            hp_aug, r_sc, ed2_sc = st["hp_aug"], st["r_sc"], st["ed2_sc"]
            lgl, lgr = st["lgl"], st["lgr"]
            es_bc, es2_bc = st["es_bc"], st["es2_bc"]

            stacked = []
            for it in range(NT):
                stt = x1_pool.tile([128, C1], BF16, tag="x1pre",
                                   name=f"x1pre_{g}_{layer}_{it}")
                stacked.append(stt)

            # ---- per head: E build + aggregation + normalize ----
            for h in range(H):
                et_h, rhs_h = [], []
                for jt in range(NT):
                    et = et_pool.tile([128, N], BF16, tag="et",
                                      name=f"et_{g}_{layer}_{h}_{jt}")
                    if _a_path(h, jt):
                        # logits + additive mask in PSUM, then 2 exps + max
                        px = ps_lg.tile([128, N], F32, tag="lg",
                                        name=f"px_{g}_{layer}_{h}_{jt}")
                        # x[j,i] = 1*s_h[i] + d_h[j]*1 + adjM[j,i]
                        nc.tensor.matmul(
                            px[:],
                            lgl[:, h * N + jt * 128:h * N + (jt + 1) * 128],
                            lgr[:, h * N:(h + 1) * N],
                            start=True, stop=False)
                        nc.tensor.matmul(
                            px[:], id_sb[:], adjm_t[jt][:],
                            start=False, stop=True)
                        t1 = u_pool.tile([128, N], BF16, tag="u",
                                         name=f"t1_{g}_{layer}_{h}_{jt}")
                        nc.scalar.activation(t1[:], px[:], AF.Exp)
                        t2 = u_pool.tile([128, N], BF16, tag="u",
                                         name=f"t2_{g}_{layer}_{h}_{jt}")
                        nc.scalar.activation(t2[:], px[:], AF.Exp, scale=0.2)
                        nc.vector.tensor_tensor(
                            et[:], t1[:], t2[:], AluOpType.max)
                        rhs_h.append(hp_aug[jt])
                    else:
                        # rank-1 exp factors + per-partition scalars
                        hps = hps_pool.tile([128, F + 1], BF16, tag="hps",
                                            name=f"hps_{g}_{layer}_{h}_{jt}")
                        nc.vector.tensor_scalar_mul(
                            hps[:], hp_aug[jt][:], ed2_sc[jt][:, h:h + 1])
                        rhs_h.append(hps)
                        w = u_pool.tile([128, N], BF16, tag="u",
                                        name=f"w_{g}_{layer}_{h}_{jt}")
                        nc.vector.tensor_scalar_mul(
                            w[:], es_bc[h], r_sc[jt][:, h:h + 1])
                        u = u_pool.tile([128, N], BF16, tag="u",
                                        name=f"u_{g}_{layer}_{h}_{jt}")
                        nc.vector.tensor_tensor(
                            u[:], w[:], es2_bc[h], AluOpType.max)
                        eng = nc.gpsimd if _gps_mask(h, jt) else nc.vector
                        eng.tensor_tensor(
                            et[:], u[:], adjp_t[jt][:], AluOpType.mult)
                    et_h.append(et)

                po = ps_agg.tile([128, NT * (F + 1)], F32, tag="agg",
                                 name=f"po_{g}_{layer}_{h}")
                for it in range(NT):
                    for jt in range(NT):
                        nc.tensor.matmul(
                            po[:, it * (F + 1):(it + 1) * (F + 1)],
                            et_h[jt][:, it * 128:(it + 1) * 128],
                            rhs_h[jt][:],
                            start=(jt == 0), stop=(jt == NT - 1))

                den = den_pool.tile([128, NT], F32, tag="den",
                                    name=f"den_{g}_{layer}_{h}")
                nc.scalar.copy(
                    den[:], po[:, F:F + 1 + (NT - 1) * (F + 1):F + 1])
                rd = den_pool.tile([128, NT], F32, tag="rd",
                                   name=f"rd_{g}_{layer}_{h}")
                nc.vector.reciprocal(rd[:], den[:])
                for it in range(NT):
                    dst = stacked[it][:, h * F:(h + 1) * F]
                    src_ = po[:, it * (F + 1):it * (F + 1) + F]
                    if (h + it) % 2 == 0:
                        nc.scalar.activation(dst, src_, AF.Copy,
                                             scale=rd[:, it:it + 1])
                    else:
                        nc.vector.tensor_scalar_mul(dst, src_,
                                                    rd[:, it:it + 1])

            # ---- post ----
            if layer == 0:
                x1t = [x1t_pool.tile([128, N], BF16, tag="x1t",
                                     name=f"x1t_{g}_{ct}")
                       for ct in range(NT)]
                graphs[g]["x1t"] = x1t
                for it in range(NT):
                    t = post_pool.tile([128, C1], BF16, tag="expt",
                                       name=f"expt_{g}_{it}")
                    nc.scalar.activation(t[:], stacked[it][:], AF.Exp)
                    u2 = post_pool.tile([128, C1], BF16, tag="u2",
                                        name=f"u2_{g}_{it}")
                    nc.vector.tensor_scalar_add(u2[:], t[:], -1.0)
                    x1e = post_pool.tile([128, C1], BF16, tag="x1e",
                                         name=f"x1e_{g}_{it}")
                    # elu(x) = min(relu(x), exp(x) - 1)
                    nc.vector.scalar_tensor_tensor(
                        x1e[:], stacked[it][:], 0.0, u2[:],
                        AluOpType.max, AluOpType.min)
                    for ct in range(NT):
                        ptp = ps_lg.tile([128, 128], BF16, tag="lg",
                                         name=f"ptp_{g}_{it}_{ct}")
                        nc.tensor.transpose(
                            ptp[:], x1e[:, ct * 128:(ct + 1) * 128], id_sb[:])
                        dst = x1t[ct][:, it * 128:(it + 1) * 128]
                        if ct % 2 == 0:
                            nc.scalar.copy(dst, ptp[:])
                        else:
                            nc.vector.tensor_copy(dst, ptp[:])
            else:
                for it in range(NT):
                    t1 = out_pool.tile([128, C1 // 2], F32, tag="t1",
                                       name=f"ot1_{g}_{it}")
                    nc.vector.tensor_add(t1[:], stacked[it][:, 0:256],
                                         stacked[it][:, 256:512])
                    t2 = out_pool.tile([128, C1 // 4], F32, tag="t2",
                                       name=f"ot2_{g}_{it}")
                    nc.vector.tensor_add(t2[:], t1[:, 0:128], t1[:, 128:256])
                    oo = out_pool.tile([128, F], F32, tag="oo",
                                       name=f"oo_{g}_{it}")
                    nc.vector.tensor_add(oo[:], t2[:, 0:64], t2[:, 64:128])
                    nc.sync.dma_start(out[g, it * 128:(it + 1) * 128, :], oo[:])

        # software-pipelined emission: unit k+1's prologue lands before
        # unit k's main body so in-order engines don't head-of-line block
        U = [(0, 0), (1, 0), (2, 0), (0, 1), (3, 0), (1, 1), (2, 1), (3, 1)]
        pending = {U[0]: prologue(*U[0])}
        for i, u in enumerate(U):
            if i + 1 < len(U):
                nxt = U[i + 1]
                pending[nxt] = prologue(*nxt)
            main(u[0], u[1], pending.pop(u))


def _get_nc():
    if "nc" not in _cached:
        _cached["nc"] = _build()
    return _cached["nc"]


def _prep_inputs(x, adj, W0, a_src0, a_dst0, W1, a_src1, a_dst1):
    x = np.asarray(x, np.float32)
    adj = np.array(adj, np.float32, copy=True)
    idx = np.arange(N)
    adj[:, idx, idx] = 1.0  # self loops (reference mutates adj the same way)
    xT = np.ascontiguousarray(x.transpose(0, 2, 1))          # [B, 64, 512]
    adjPf = np.where(adj > 0, np.float32(1), np.float32(0)).astype(BF)
    adjMf = np.where(adj > 0, np.float32(0), np.float32(-BIG)).astype(BF)
    identf = np.eye(128, dtype=np.float32).astype(BF)
    W0 = np.asarray(W0, np.float32)
    W1 = np.asarray(W1, np.float32)
    w0d = np.concatenate(
        [W0, W0 @ np.asarray(a_dst0, np.float32),
         W0 @ np.asarray(a_src0, np.float32)], axis=1)
    w1d = np.concatenate(
        [W1, W1 @ np.asarray(a_dst1, np.float32),
         W1 @ np.asarray(a_src1, np.float32)], axis=1).astype(BF)
    in_maps = []
    for c in range(NCORES):
        sl = slice(c * G, (c + 1) * G)
        in_maps.append(dict(
            xT=np.ascontiguousarray(xT[sl]),
            adjP=np.ascontiguousarray(adjPf[sl]),
            adjM=np.ascontiguousarray(adjMf[sl]),
            ident=identf, w0d=w0d, w1d=w1d,
        ))
    return in_maps


def run(inputs, **kw):
    """Build+run; returns (output [B,N,F] float32, BassKernelResults)."""
    nc = _get_nc()
    in_maps = _prep_inputs(
        inputs["x"], inputs["adj"], inputs["W0"], inputs["a_src0"],
        inputs["a_dst0"], inputs["W1"], inputs["a_src1"], inputs["a_dst1"])
    res = run_bass_kernel_spmd(nc, in_maps, list(range(NCORES)), **kw)
    outs = [res.results[c]["out"].reshape(G, N, F) for c in range(NCORES)]
    return np.concatenate(outs, axis=0).astype(np.float32), res


def kernel(**inputs):
    out, _ = run(inputs)
    return out



# revision 5
# speedup vs baseline: 1.2517x; 1.2517x over previous
"""BatchGAT (2-layer, 8-head GAT over 32 graphs of 512 nodes) on 8 TRN2 NeuronCores.

Data-parallel over the batch: each core processes 4 graphs. Per graph/layer the
masked attention matrix is built in transposed layout E^T[j, i] (j = neighbor on
partitions) and the aggregation runs TRANSPOSED on TensorE: lhsT = hp_aug
([128, 65] stationary, col 64 = const), rhs = E^T ([128, 1024], two heads side
by side) -> po[65, 1024] PSUM, so the output lands FEATURE-major - exactly the
lhsT layout the next layer's projection needs (no transposes), and the softmax
denominator falls out as PSUM row 64.

E-build exploits softmax row-scale invariance: dividing row i of E by
exp(0.2*s_i) gives E'' = max(q_i * ed_j, ed2_j) * adj with q = exp(0.8 s)
(a broadcast row tile) and ed = exp(d), ed2 = exp(0.2 d) per-partition scalars.
That is ONE dual-op tensor_scalar (mult-ptr, max-ptr) in 4x DVE mode per
(head, jtile), plus one mask multiply (tensor_tensor, split DVE/GpSimd).

Normalization: den (PSUM row 64) is evacuated as part of a 65-row ACT copy
(free - ACT time is free-dim bound), gathered by tiny SBUF DMAs, reciprocal'd
once per graph-layer [8, 512], bounced via DRAM and partition-broadcast into
[128, 512] tiles for a single tensor_tensor multiply per c-tile. The layer-1
head mean (/8) is folded into the aug column (8.0); the head sum runs on
TensorE with a [I64; I64] selector rhs, emitting node-major output directly.
"""

import sys

if "/opt/trn_rl_repo" not in sys.path:
    sys.path.insert(0, "/opt/trn_rl_repo")

import numpy as np
import ml_dtypes

import concourse.bacc as bacc
import concourse.mybir as mybir
from concourse import tile
from concourse.bass_utils import run_bass_kernel_spmd
from concourse.alu_op_type import AluOpType

F32 = mybir.dt.float32
BF16 = mybir.dt.bfloat16
BF = ml_dtypes.bfloat16
AF = mybir.ActivationFunctionType

B, N, FIN, H, F = 32, 512, 64, 8, 64
NCORES = 8
G = B // NCORES          # graphs per core
NT = N // 128            # node tiles
C1 = H * F               # layer-1 input features (512)
W2 = 2 * N               # two heads side by side

_cached = {}


def _mask_gps(w, jt):
    """Which (wave, jtile) mask multiplies run on GpSimd instead of DVE."""
    return (w * NT + jt) % 16 in (1, 4, 7, 10, 13)


def _build():
    nc = bacc.Bacc("TRN2", target_bir_lowering=False, debug=False)

    xT = nc.dram_tensor("xT", [G, FIN, N], F32, kind="ExternalInput").ap()
    adjP = nc.dram_tensor("adjP", [G, N, N], BF16, kind="ExternalInput").ap()  # {0,1}
    w0d = nc.dram_tensor("w0d", [FIN, F + 2 * H], F32, kind="ExternalInput").ap()
    w1d = nc.dram_tensor("w1d", [C1, F + 2 * H], BF16, kind="ExternalInput").ap()
    sel2 = nc.dram_tensor("sel2", [128, F], BF16, kind="ExternalInput").ap()
    out = nc.dram_tensor("out", [G, N, F], F32, kind="ExternalOutput").ap()

    with tile.TileContext(nc) as tc:
        _emit(nc, tc, xT, adjP, w0d, w1d, sel2, out)
    nc.compile()
    return nc


def _emit(nc, tc, xT, adjP, w0d, w1d, sel2, out):
    from contextlib import ExitStack

    ctx = ExitStack()
    with ctx:
        # weights: [W | W@a_dst | W@a_src] -> proj matmul yields [hp | d | s]
        wpool = ctx.enter_context(tc.tile_pool(name="weights", bufs=1))
        w0d_sb = wpool.tile([FIN, F + 2 * H], F32, tag="w0d")
        nc.sync.dma_start(w0d_sb[:], w0d[:])
        w1d_sb = wpool.tile([128, NT, F + 2 * H], BF16, tag="w1d")
        nc.sync.dma_start(w1d_sb[:], w1d.rearrange("(c p) f -> p c f", p=128))
        sel2_sb = wpool.tile([128, F], BF16, tag="sel2")
        nc.sync.dma_start(sel2_sb[:], sel2[:])

        xt_pool = ctx.enter_context(tc.tile_pool(name="xt", bufs=3))
        adj_pool = ctx.enter_context(tc.tile_pool(name="adj", bufs=4 * NT))
        hp_pool = ctx.enter_context(tc.tile_pool(name="hp", bufs=3 * NT))
        dsc_pool = ctx.enter_context(tc.tile_pool(name="dscal", bufs=3 * NT))
        qrow_pool = ctx.enter_context(tc.tile_pool(name="qrow", bufs=3))
        qbc_pool = ctx.enter_context(tc.tile_pool(name="qbc", bufs=2 * NT))
        u_pool = ctx.enter_context(tc.tile_pool(name="u", bufs=6))
        et_pool = ctx.enter_context(tc.tile_pool(name="et", bufs=2 * NT))
        xe_pool = ctx.enter_context(tc.tile_pool(name="xe", bufs=2 * NT))
        scr_pool = ctx.enter_context(tc.tile_pool(name="scr", bufs=4))
        den_pool = ctx.enter_context(tc.tile_pool(name="den", bufs=4))
        rdbc_pool = ctx.enter_context(tc.tile_pool(name="rdbc", bufs=2 * NT))
        x1t_pool = ctx.enter_context(tc.tile_pool(name="x1t", bufs=4 * NT))
        x2t_pool = ctx.enter_context(tc.tile_pool(name="x2t", bufs=2 * NT))
        post_pool = ctx.enter_context(tc.tile_pool(name="post", bufs=4))
        out_pool = ctx.enter_context(tc.tile_pool(name="out", bufs=4))
        dbnc_pool = ctx.enter_context(tc.tile_pool(name="dbnc", bufs=3, space="DRAM"))

        ps_proj = ctx.enter_context(tc.tile_pool(name="ps_proj", bufs=2, space="PSUM"))
        ps_s = ctx.enter_context(tc.tile_pool(name="ps_s", bufs=1, space="PSUM"))
        ps_agg = ctx.enter_context(tc.tile_pool(name="ps_agg", bufs=2, space="PSUM"))
        ps_out = ctx.enter_context(tc.tile_pool(name="ps_out", bufs=1, space="PSUM"))

        graphs = {}

        def prologue(g, layer):
            st = {}
            if layer == 0:
                xt = xt_pool.tile([FIN, N], F32, tag="xt", name=f"xt_{g}")
                nc.gpsimd.dma_start(xt[:], xT[g])
                adj2 = []
                for jt in range(NT):
                    a2 = adj_pool.tile([128, W2], BF16, tag="adj2",
                                       name=f"adj2_{g}_{jt}")
                    nc.gpsimd.dma_start(
                        a2[:, 0:N], adjP[g, jt * 128:(jt + 1) * 128, :])
                    nc.gpsimd.dma_start(
                        a2[:, N:W2], adjP[g, jt * 128:(jt + 1) * 128, :])
                    adj2.append(a2)
                graphs[g] = dict(xt=xt, adj2=adj2)
            gs = graphs[g]
            xt = gs["xt"]
            x1t = gs.get("x1t")

            # ---- projections: [hp | d | s] per node tile ----
            hp_aug, ed_sc, ed2_sc = [], [], []
            for jt in range(NT):
                pp = ps_proj.tile([128, F + 2 * H], F32, tag="proj",
                                  name=f"pp_{g}_{layer}_{jt}")
                if layer == 0:
                    nc.tensor.matmul(
                        pp[:], xt[:, jt * 128:(jt + 1) * 128], w0d_sb[:],
                        start=True, stop=True)
                else:
                    for ct in range(NT):
                        nc.tensor.matmul(
                            pp[:], x1t[ct][:, jt * 128:(jt + 1) * 128],
                            w1d_sb[:, ct, :],
                            start=(ct == 0), stop=(ct == NT - 1))
                ha = hp_pool.tile([128, F + 1], BF16, tag="hp",
                                  name=f"ha_{g}_{layer}_{jt}")
                nc.scalar.copy(ha[:, 0:F], pp[:, 0:F])
                # aug column: 8.0 on layer 1 folds the head-mean into 1/den
                nc.vector.memset(ha[:, F:F + 1], 1.0 if layer == 0 else 8.0)
                hp_aug.append(ha)
                ee = dsc_pool.tile([128, 2 * H], F32, tag="edsc",
                                   name=f"ee_{g}_{layer}_{jt}")
                nc.scalar.activation(ee[:, 0:H], pp[:, F:F + H], AF.Exp,
                                     scale=1.0)
                nc.scalar.activation(ee[:, H:2 * H], pp[:, F:F + H], AF.Exp,
                                     scale=0.2)
                ed_sc.append(ee[:, 0:H])
                ed2_sc.append(ee[:, H:2 * H])

            # s rows -> q = exp(0.8 s) -> DRAM bounce -> broadcast per wave
            psd = ps_s.tile([H, N], F32, tag="s", name=f"psd_{g}_{layer}")
            if layer == 0:
                nc.tensor.matmul(psd[:], w0d_sb[:, F + H:F + 2 * H], xt[:],
                                 start=True, stop=True)
            else:
                for ct in range(NT):
                    nc.tensor.matmul(
                        psd[:], w1d_sb[:, ct, F + H:F + 2 * H], x1t[ct][:],
                        start=(ct == 0), stop=(ct == NT - 1))
            qd = qrow_pool.tile([H, N], BF16, tag="qd",
                                name=f"qd_{g}_{layer}")
            nc.scalar.activation(qd[:], psd[:], AF.Exp, scale=0.8)
            qdram = dbnc_pool.tile([H, N], BF16, tag="qdram",
                                   name=f"qdram_{g}_{layer}")
            nc.sync.dma_start(qdram[:], qd[:])
            q_bc = []
            for w in range(NT):
                qb = qbc_pool.tile([128, W2], BF16, tag="qbc",
                                   name=f"qbc_{g}_{layer}_{w}")
                nc.sync.dma_start(
                    qb[:],
                    qdram[2 * w:2 * w + 2, :]
                    .rearrange("a f -> () (a f)").partition_broadcast(128))
                q_bc.append(qb)

            st.update(hp_aug=hp_aug, ed_sc=ed_sc, ed2_sc=ed2_sc, q_bc=q_bc)
            return st

        def main(g, layer, st):
            gs = graphs[g]
            adj2 = gs["adj2"]
            hp_aug, ed_sc, ed2_sc = st["hp_aug"], st["ed_sc"], st["ed2_sc"]
            q_bc = st["q_bc"]

            den8 = den_pool.tile([H, N], BF16, tag="den8",
                                 name=f"den8_{g}_{layer}")
            xe = []
            # ---- per wave (2 heads): E build + aggregation + evac ----
            for w in range(NT):
                h0, h1 = 2 * w, 2 * w + 1
                et_w = []
                for jt in range(NT):
                    u2 = u_pool.tile([128, W2], BF16, tag="u",
                                     name=f"u_{g}_{layer}_{w}_{jt}")
                    # E'' = max(q_i * ed_j, ed2_j), one 4x TSP per head
                    nc.vector.tensor_scalar(
                        u2[:, 0:N], q_bc[w][:, 0:N],
                        ed_sc[jt][:, h0:h0 + 1], ed2_sc[jt][:, h0:h0 + 1],
                        AluOpType.mult, AluOpType.max)
                    nc.vector.tensor_scalar(
                        u2[:, N:W2], q_bc[w][:, N:W2],
                        ed_sc[jt][:, h1:h1 + 1], ed2_sc[jt][:, h1:h1 + 1],
                        AluOpType.mult, AluOpType.max)
                    et = et_pool.tile([128, W2], BF16, tag="et",
                                      name=f"et_{g}_{layer}_{w}_{jt}")
                    eng = nc.gpsimd if _mask_gps(w, jt) else nc.vector
                    eng.tensor_tensor(et[:], u2[:], adj2[jt][:],
                                      AluOpType.mult)
                    et_w.append(et)

                po = ps_agg.tile([F + 1, W2], F32, tag="agg",
                                 name=f"po_{g}_{layer}_{w}")
                for jt in range(NT):
                    nc.tensor.matmul(po[:, 0:N], hp_aug[jt][:],
                                     et_w[jt][:, 0:N],
                                     start=(jt == 0), stop=(jt == NT - 1))
                for jt in range(NT):
                    nc.tensor.matmul(po[:, N:W2], hp_aug[jt][:],
                                     et_w[jt][:, N:W2],
                                     start=(jt == 0), stop=(jt == NT - 1))

                # evac: 65 rows (den rides along as row 64, free on ACT)
                xew = xe_pool.tile([128, N], BF16, tag="xe",
                                   name=f"xe_{g}_{layer}_{w}")
                nc.scalar.copy(xew[0:F + 1, :], po[0:F + 1, 0:N])
                nc.sync.dma_start(den8[h0:h0 + 1, :], xew[F:F + 1, :])
                scr = scr_pool.tile([F + 1, N], BF16, tag="scr",
                                    name=f"scr_{g}_{layer}_{w}")
                nc.scalar.copy(scr[:], po[0:F + 1, N:W2])
                nc.sync.dma_start(den8[h1:h1 + 1, :], scr[F:F + 1, :])
                nc.sync.dma_start(xew[F:128, :], scr[0:F, :])
                xe.append(xew)

            # ---- normalization scales ----
            rdrow = den_pool.tile([H, N], BF16, tag="rdrow",
                                  name=f"rd_{g}_{layer}")
            with nc.allow_low_precision(reason="1/den in bf16 is ample"):
                nc.vector.reciprocal(rdrow[:], den8[:])
            rddram = dbnc_pool.tile([H, N], BF16, tag="rddram",
                                    name=f"rddram_{g}_{layer}")
            nc.sync.dma_start(rddram[:], rdrow[:])
            rdbc = []
            for ct in range(NT):
                rb = rdbc_pool.tile([128, N], BF16, tag="rdbc",
                                    name=f"rdbc_{g}_{layer}_{ct}")
                nc.sync.dma_start(
                    rb[0:F, :],
                    rddram[2 * ct:2 * ct + 1, :].partition_broadcast(F))
                nc.sync.dma_start(
                    rb[F:128, :],
                    rddram[2 * ct + 1:2 * ct + 2, :].partition_broadcast(F))
                rdbc.append(rb)

            # ---- post ----
            if layer == 0:
                x1t = [x1t_pool.tile([128, N], BF16, tag="x1t",
                                     name=f"x1t_{g}_{ct}")
                       for ct in range(NT)]
                graphs[g]["x1t"] = x1t
                for ct in range(NT):
                    xn = post_pool.tile([128, N], BF16, tag="xn",
                                        name=f"xn_{g}_{ct}")
                    nc.vector.tensor_tensor(xn[:], xe[ct][:], rdbc[ct][:],
                                            AluOpType.mult)
                    te = post_pool.tile([128, N], BF16, tag="te",
                                        name=f"te_{g}_{ct}")
                    nc.scalar.activation(te[:], xn[:], AF.Exp)
                    rl = post_pool.tile([128, N], BF16, tag="rl",
                                        name=f"rl_{g}_{ct}")
                    nc.vector.tensor_scalar(rl[:], xn[:], 0.0, None,
                                            AluOpType.max)
                    # elu(x) = min(relu(x), exp(x) - 1)
                    nc.vector.scalar_tensor_tensor(
                        x1t[ct][:], te[:], -1.0, rl[:],
                        AluOpType.add, AluOpType.min)
            else:
                x2t = []
                for ct in range(NT):
                    xn = x2t_pool.tile([128, N], BF16, tag="x2t",
                                       name=f"x2t_{g}_{ct}")
                    nc.vector.tensor_tensor(xn[:], xe[ct][:], rdbc[ct][:],
                                            AluOpType.mult)
                    x2t.append(xn)
                for it in range(NT):
                    po2 = ps_out.tile([128, F], F32, tag="out",
                                      name=f"po2_{g}_{it}")
                    for ct in range(NT):
                        nc.tensor.matmul(
                            po2[:], x2t[ct][:, it * 128:(it + 1) * 128],
                            sel2_sb[:], start=(ct == 0), stop=(ct == NT - 1))
                    oo = out_pool.tile([128, F], F32, tag="oo",
                                       name=f"oo_{g}_{it}")
                    nc.scalar.copy(oo[:], po2[:])
                    nc.sync.dma_start(out[g, it * 128:(it + 1) * 128, :],
                                      oo[:])

        # software-pipelined emission: unit k+1's prologue lands before
        # unit k's main body so in-order engines don't head-of-line block
        U = [(0, 0), (1, 0), (2, 0), (0, 1), (3, 0), (1, 1), (2, 1), (3, 1)]
        pending = {U[0]: prologue(*U[0])}
        for i, u in enumerate(U):
            if i + 1 < len(U):
                nxt = U[i + 1]
                pending[nxt] = prologue(*nxt)
            main(u[0], u[1], pending.pop(u))


def _get_nc():
    if "nc" not in _cached:
        _cached["nc"] = _build()
    return _cached["nc"]


def _prep_inputs(x, adj, W0, a_src0, a_dst0, W1, a_src1, a_dst1):
    x = np.asarray(x, np.float32)
    adj = np.array(adj, np.float32, copy=True)
    idx = np.arange(N)
    adj[:, idx, idx] = 1.0  # self loops (reference mutates adj the same way)
    xT = np.ascontiguousarray(x.transpose(0, 2, 1))          # [B, 64, 512]
    adjPf = np.where(adj > 0, np.float32(1), np.float32(0)).astype(BF)
    W0 = np.asarray(W0, np.float32)
    W1 = np.asarray(W1, np.float32)
    w0d = np.concatenate(
        [W0, W0 @ np.asarray(a_dst0, np.float32),
         W0 @ np.asarray(a_src0, np.float32)], axis=1)
    w1d = np.concatenate(
        [W1, W1 @ np.asarray(a_dst1, np.float32),
         W1 @ np.asarray(a_src1, np.float32)], axis=1).astype(BF)
    sel2f = np.tile(np.eye(F, dtype=np.float32), (2, 1)).astype(BF)
    in_maps = []
    for c in range(NCORES):
        sl = slice(c * G, (c + 1) * G)
        in_maps.append(dict(
            xT=np.ascontiguousarray(xT[sl]),
            adjP=np.ascontiguousarray(adjPf[sl]),
            w0d=w0d, w1d=w1d, sel2=sel2f,
        ))
    return in_maps


def run(inputs, **kw):
    """Build+run; returns (output [B,N,F] float32, BassKernelResults)."""
    nc = _get_nc()
    in_maps = _prep_inputs(
        inputs["x"], inputs["adj"], inputs["W0"], inputs["a_src0"],
        inputs["a_dst0"], inputs["W1"], inputs["a_src1"], inputs["a_dst1"])
    res = run_bass_kernel_spmd(nc, in_maps, list(range(NCORES)), **kw)
    outs = [res.results[c]["out"].reshape(G, N, F) for c in range(NCORES)]
    return np.concatenate(outs, axis=0).astype(np.float32), res


def kernel(**inputs):
    out, _ = run(inputs)
    return out
